# revision 22
# baseline (speedup 1.0000x reference)
"""Trainium2 Bass kernel for nn_BioV_19748259627109.

Pipeline per core (data-parallel over batch B=8, one sample per core):
  S1  spatial 3x3 conv (1->3ch) as PE band-matmuls over H (f32r) in
      (w,t)-inner order, SiLU -> bf16 [h,(c,w,t)]
  EX  h<->t exchange as contiguous 32x32 DVE stream transposes per c
      (t-inner layout makes blocks (t x i) with w preserved), overlapped
      with the next c's matmuls; S2 per-c starts right after its c
  S2  temporal depthwise conv (7 taps) as block-diag bf16 PE matmuls
      with fused silu/silu + per-partition stat sidebands
  AR  stats partition-reduced on PE (ones-matmul), scalar math on the
      otherwise-idle GpSimd queue feeding the collective trigger;
      kv contractions overlap the AllReduce
  KV  kv_s via block-diag PE contraction over t; kv_t via DVE dot rows
  OUT max-free softmax (inputs bounded), PE row-broadcasts, softmax
      rsqrt factors folded into the 96-element At row, rank-1 outer
      product on DVE/GPSIMD, DMA'd straight to HBM

out[c,t,s] = At[c,t]*As[c,s] exactly; SwitchNorm is affine per (b,c) and
commutes with the kv contractions, so xn is never materialized."""
import sys
if '/opt/trn_rl_repo' not in sys.path:
    sys.path.insert(0, '/opt/trn_rl_repo')

import numpy as np
from concourse import bass, bacc, tile, mybir

F32 = mybir.dt.float32
F32R = mybir.dt.float32r
BF16 = mybir.dt.bfloat16
BF16_NP = mybir.dt.np(BF16)
ALU = mybir.AluOpType
AFT = mybir.ActivationFunctionType
AXT = mybir.AxisListType

N_CORES = 8
B, T, H, W = 8, 32, 128, 128
C = 3
NTOT = float(T * H * W)
EPS = 1e-5
S1_BF16 = False


def _host_constants(inputs):
    w_s = np.asarray(inputs['w_spatial'], np.float32)     # (3,1,3,3)
    b_s = np.asarray(inputs['b_spatial'], np.float32)
    w_t = np.asarray(inputs['w_temporal'], np.float32)    # (3,1,7,1)
    b_t = np.asarray(inputs['b_temporal'], np.float32)
    sn_w = np.asarray(inputs['sn_weight'], np.float32).reshape(3)
    sn_b = np.asarray(inputs['sn_bias'], np.float32).reshape(3)
    mwr = np.asarray(inputs['mean_weight'], np.float32)
    vwr = np.asarray(inputs['var_weight'], np.float32)
    mw = np.exp(mwr - mwr.max()); mw = mw / mw.sum()
    vw = np.exp(vwr - vwr.max()); vw = vw / vw.sum()
    wkvs = np.asarray(inputs['w_kv_s'], np.float32)       # (2,32)
    wkvt = np.asarray(inputs['w_kv_t'], np.float32)       # (2,16384)

    # bandW[h_in, c, dx, h_out] = w_s[c,0,h_in-h_out+1,dx]
    hi = np.arange(128)[:, None]
    ho = np.arange(128)[None, :]
    dy = hi - ho + 1
    bandw = np.zeros((128, 3, 3, 128), np.float32)
    for c in range(3):
        for dx in range(3):
            m = np.where((dy >= 0) & (dy <= 2), w_s[c, 0, np.clip(dy, 0, 2), dx], 0.0)
            bandw[:, c, dx, :] = m.astype(np.float32)
    if S1_BF16:
        bandw = bandw.astype(BF16_NP)

    # bandT[(q,t_in), c, (q,t_out)] block-diagonal over h-quarters, bf16
    ti = np.arange(32)[:, None]
    to = np.arange(32)[None, :]
    kk = ti - to + 3
    bandt32 = np.zeros((32, 3, 32), np.float32)
    for c in range(3):
        bandt32[:, c, :] = np.where((kk >= 0) & (kk <= 6), w_t[c, 0, np.clip(kk, 0, 6), 0], 0.0)
    bandt = np.zeros((128, 3, 128), np.float32)
    for q in range(4):
        bandt[32 * q:32 * q + 32, :, 32 * q:32 * q + 32] = bandt32
    bandt = bandt.astype(BF16_NP)

    # kv_s lhsT [(q,t)=128, (o,q0)=8] -- o-major so evac rows are contiguous
    kvs_lhst = np.zeros((128, 8), np.float32)
    for q in range(4):
        for t in range(32):
            for o in range(2):
                kvs_lhst[q * 32 + t, o * 4 + q] = wkvs[o, t]
    kvs_lhst = kvs_lhst.astype(BF16_NP)

    qsum = np.zeros((128, 32), np.float32)
    qsum[np.arange(128), np.arange(128) % 32] = 1.0

    # wkvt in (q, o, w, i) order to match gB's (w, i) free layout
    wkvt4 = wkvt.reshape(2, 4, 32, 128).transpose(1, 0, 3, 2).astype(BF16_NP)[None]

    ws_sum = wkvs.sum(axis=1)   # (2,)
    wt_sum = wkvt.sum(axis=1)   # (2,)
    # crow layout: [0:3] sn_w, [3:6] sn_b, [6:12] Ws[o] in (c,o) order,
    # [12:18] Wt[o] in (o,c) order
    crow = np.zeros((1, 32), np.float32)
    crow[0, 0:3] = sn_w
    crow[0, 3:6] = sn_b
    crow[0, 6:12] = np.tile(ws_sum, 3)
    crow[0, 12:18] = np.repeat(wt_sum, 3)
    scal = dict(
        b_s=[float(v) for v in b_s], b_t=[float(v) for v in b_t],
        mw=[float(v) for v in mw], vw=[float(v) for v in vw],
    )
    return dict(bandw=bandw, bandt=bandt, kvs_lhst=kvs_lhst, qsum=qsum,
                wkvt4=wkvt4, crow=crow, scal=scal)


def build_program(scal, no_cc=False):
    nc = bacc.Bacc("TRN2", target_bir_lowering=False, debug=False,
                   num_devices=N_CORES)

    xdt = BF16 if S1_BF16 else F32R
    # x in [h, w(+halo), t] order: t innermost everywhere
    xin = nc.dram_tensor("xin", [128, 130, 32], xdt, kind="ExternalInput")
    bandw_d = nc.dram_tensor("bandw", [128, 3, 3, 128], xdt, kind="ExternalInput")
    bandt_d = nc.dram_tensor("bandt", [128, 3, 128], BF16, kind="ExternalInput")
    kvsl_d = nc.dram_tensor("kvs_lhst", [128, 8], BF16, kind="ExternalInput")
    qsum_d = nc.dram_tensor("qsum", [128, 32], F32, kind="ExternalInput")
    wkvt_d = nc.dram_tensor("wkvt4", [1, 4, 2, 128, 32], BF16, kind="ExternalInput")
    crow_d = nc.dram_tensor("crow", [1, 32], F32, kind="ExternalInput")
    out_d = nc.dram_tensor("out", [3, 32, 128, 128], F32, kind="ExternalOutput")

    b_s, b_t = scal['b_s'], scal['b_t']
    mw, vw = scal['mw'], scal['vw']

    with tile.TileContext(nc) as tc:
        with (
            tc.tile_pool(name="const", bufs=1) as cpool,
            tc.tile_pool(name="big", bufs=1) as bigp,
            tc.tile_pool(name="work", bufs=2) as wpool,
            tc.tile_pool(name="outw", bufs=4) as opool,
            tc.tile_pool(name="psum", bufs=2, space="PSUM") as pp,
            tc.tile_pool(name="dram", bufs=1, space="DRAM") as dram,
        ):
            # ---- loads ordered for earliest S1 start ----
            bandw_sb = cpool.tile([128, 3, 3, 128], xdt)
            x_sb = bigp.tile([128, 130, 32], xdt, tag="xbig")
            nc.sync.dma_start(x_sb[:, 0:34, :], xin[:, 0:34, :])
            nc.sync.dma_start(bandw_sb[:, 0], bandw_d[:, 0])
            nc.sync.dma_start(x_sb[:, 34:82, :], xin[:, 34:82, :])
            nc.sync.dma_start(x_sb[:, 82:130, :], xin[:, 82:130, :])
            nc.sync.dma_start(bandw_sb[:, 1], bandw_d[:, 1])
            nc.sync.dma_start(bandw_sb[:, 2], bandw_d[:, 2])
            bandt_sb = cpool.tile([128, 3, 128], BF16)
            nc.scalar.dma_start(bandt_sb[:], bandt_d[:])
            kvsl_sb = cpool.tile([128, 8], BF16)
            nc.scalar.dma_start(kvsl_sb[:], kvsl_d[:])
            qsum_sb = cpool.tile([128, 32], F32)
            nc.scalar.dma_start(qsum_sb[:], qsum_d[:])
            crow_sb = cpool.tile([1, 32], F32)
            nc.scalar.dma_start(crow_sb[:], crow_d[:])
            bvals = cpool.tile([128, 8], F32)
            for c in range(3):
                nc.vector.memset(bvals[:, c:c + 1], b_s[c])
                nc.vector.memset(bvals[:, 3 + c:4 + c], b_t[c])
            nc.vector.memset(bvals[:, 6:7], EPS)
            ones_col = cpool.tile([128, 1], F32)
            nc.gpsimd.memset(ones_col[:], 1.0)
            ones_row = cpool.tile([1, 128], F32)
            nc.gpsimd.memset(ones_row[:], 1.0)
            sc = cpool.tile([1, 32], F32)
            nc.gpsimd.memset(sc[:, 22:24], 0.0)
            cst = cpool.tile([1, 12], F32)
            nc.gpsimd.memset(cst[:, 0:3], 1.0 / NTOT)
            nc.gpsimd.memset(cst[:, 3:6], NTOT / (NTOT - 1.0))
            nc.gpsimd.memset(cst[:, 6:8], 1.0 / 3.0)
            nc.gpsimd.memset(cst[:, 8:9], mw[1])
            nc.gpsimd.memset(cst[:, 9:10], vw[1])

            yS1 = bigp.tile([128, 3, 128, 32], BF16, tag="ys1")   # [h,(c,w,t)]
            yB = bigp.tile([128, 3, 128, 32], BF16)               # [(q,t),(c,w,i)]
            gB = bigp.tile([128, 3, 128, 32], BF16)               # [(q,t),(c,w,i)]
            accs = cpool.tile([128, 12], F32)
            wkvt_sb = cpool.tile([128, 2, 128, 32], BF16)
            sq_scratch = cpool.tile([128, 4096], BF16)

            # ---- S1 + exchange + S2, pipelined per c ----
            for c in range(3):
                for wc in range(4):
                    w0 = 32 * wc
                    ps = pp.tile([128, 1024], F32, tag="mm1")
                    for j in range(2):
                        for dx in range(3):
                            nc.tensor.matmul(
                                ps[:, 512 * j:512 * (j + 1)],
                                lhsT=bandw_sb[:, c, dx, :],
                                rhs=x_sb[:, w0 + 16 * j + dx:w0 + 16 * j + dx + 16, :],
                                start=(dx == 0), stop=(dx == 2),
                            )
                    nc.scalar.activation(
                        yS1[:, c, w0:w0 + 32, :].rearrange("p a b -> p (a b)"),
                        ps[:], AFT.Silu, bias=bvals[:, c:c + 1])
                    if wc % 2 == 1:
                        h0 = 64 * (wc // 2)
                        nc.vector.transpose(
                            yB[:, c, h0:h0 + 64].rearrange("p a b -> p (a b)"),
                            yS1[:, c, h0:h0 + 64].rearrange("p a b -> p (a b)"))

                # S2 for this c: temporal conv + silu2/silu3 + stat sidebands
                zfull = wpool.tile([128, 4096], BF16, tag="zch")
                for wh in range(4):
                    w0 = 32 * wh
                    ps = pp.tile([128, 1024], F32, tag="mm2")
                    for j in range(2):
                        nc.tensor.matmul(
                            ps[:, 512 * j:512 * (j + 1)],
                            lhsT=bandt_sb[:, c, :],
                            rhs=yB[:, c, w0 + 16 * j:w0 + 16 * j + 16, :],
                            start=True, stop=True,
                        )
                    nc.scalar.activation(zfull[:, 1024 * wh:1024 * (wh + 1)],
                                         ps[:], AFT.Silu,
                                         bias=bvals[:, 3 + c:4 + c])
                    if wh % 2 == 1:
                        hh = wh // 2
                        gsl = gB[:, c, 64 * hh:64 * hh + 64, :].rearrange(
                            "p a b -> p (a b)")
                        nc.scalar.activation(
                            gsl, zfull[:, 2048 * hh:2048 * (hh + 1)], AFT.Silu,
                            accum_out=accs[:, 2 * c + hh:2 * c + hh + 1])
                        nc.vector.scalar_tensor_tensor(
                            sq_scratch[:, 0:2048], gsl, 1.0, gsl,
                            ALU.mult, ALU.mult,
                            accum_out=accs[:, 6 + 2 * c + hh:7 + 2 * c + hh])
                if c == 0:
                    # tiny yS1-dependent write into wkvt_sb gates the wkvt
                    # broadcast (WAW) off the startup window, where its
                    # packets would starve the input loads
                    for q in range(4):
                        nc.gpsimd.tensor_copy(
                            wkvt_sb[32 * q:32 * q + 1, 0, 0, 0:1],
                            yS1[32 * q:32 * q + 1, 0, 0, 0:1])

            # wkvt broadcast: needed only for kv_t (gated, see above)
            for q in range(4):
                nc.gpsimd.dma_start(
                    wkvt_sb[32 * q:32 * q + 32, :, :, :],
                    wkvt_d[0, q].unsqueeze(0).broadcast_to([32, 2, 128, 32]),
                )

            # ---- stats -> AllReduce trigger; math on idle GpSimd queue ----
            sc2 = cpool.tile([1, 32], F32)
            cc_in = dram.tile([1, 8], F32)
            cc_out = dram.tile([1, 8], F32)
            with tc.high_priority():
                ps_st = pp.tile([1, 12], F32, tag="mm2")
                nc.tensor.matmul(ps_st[:], lhsT=ones_col[:], rhs=accs[:],
                                 start=True, stop=True)
                st12 = cpool.tile([1, 12], F32)
                nc.scalar.copy(st12[:], ps_st[:])
                # halves-add; mean_in [16:19], E2 [3:6], msq [6:9],
                # var_in [9:12], temp [19:22]
                nc.gpsimd.tensor_add(sc[:, 26:29], st12[0:1, 0:6:2],
                                     st12[0:1, 1:6:2])
                nc.gpsimd.tensor_add(sc[:, 29:32], st12[0:1, 6:12:2],
                                     st12[0:1, 7:12:2])
                nc.gpsimd.tensor_mul(sc[:, 16:19], sc[:, 26:29], cst[:, 0:3])
                nc.gpsimd.tensor_mul(sc[:, 3:6], sc[:, 29:32], cst[:, 0:3])
                nc.gpsimd.tensor_mul(sc[:, 6:9], sc[:, 16:19], sc[:, 16:19])
                nc.gpsimd.tensor_sub(sc[:, 9:12], sc[:, 3:6], sc[:, 6:9])
                nc.gpsimd.tensor_mul(sc[:, 9:12], sc[:, 9:12], cst[:, 3:6])
                nc.gpsimd.tensor_add(sc[:, 19:22], sc[:, 9:12], sc[:, 6:9])
                nc.sync.dma_start(cc_in[:], sc[:, 16:24])
                if no_cc:
                    nc.sync.dma_start(cc_out[:], cc_in[:])
                else:
                    nc.gpsimd.collective_compute(
                        "AllReduce", ALU.add,
                        replica_groups=[list(range(N_CORES))],
                        ins=[cc_in.opt()], outs=[cc_out.opt()])
                nc.sync.dma_start(sc[:, 24:32], cc_out[:])
                # local layer stats during the AR window (gpsimd, off-path)
                nc.gpsimd.tensor_add(sc2[:, 12:14], sc[:, 16:20:3],
                                     sc[:, 17:21:3])
                nc.gpsimd.tensor_add(sc2[:, 12:14], sc2[:, 12:14],
                                     sc[:, 18:22:3])
                nc.gpsimd.tensor_mul(sc2[:, 12:14], sc2[:, 12:14],
                                     cst[:, 6:8])
                nc.gpsimd.tensor_mul(sc2[:, 14:15], sc2[:, 12:13], sc2[:, 12:13])
                nc.gpsimd.tensor_sub(sc2[:, 15:16], sc2[:, 13:14], sc2[:, 14:15])
                nc.gpsimd.tensor_mul(sc2[:, 26:27], sc2[:, 12:13],
                                     cst[:, 8:9])
                nc.gpsimd.tensor_mul(sc2[:, 27:28], sc2[:, 15:16],
                                     cst[:, 9:10])

            # ---- kv_s contraction (PE) + evac (scalar) + scatter ----
            kvs_tmp = bigp.tile([8, 4160], F32, tag="kvstmp")
            kvsA = cpool.tile([128, 3, 2, 128], F32)
            for c in range(3):
                gBr = gB[:, c].rearrange("p w i -> p i w")
                for ih in range(4):
                    i0 = 8 * ih
                    ps = pp.tile([8, 1024], F32, tag="mm2")
                    for j in range(2):
                        nc.tensor.matmul(
                            ps[:, 512 * j:512 * (j + 1)],
                            lhsT=kvsl_sb[:],
                            rhs=gBr[:, i0 + 4 * j:i0 + 4 * j + 4, :],
                            start=True, stop=True)
                    nc.scalar.copy(kvs_tmp[:, 1024 * ih:1024 * (ih + 1)], ps[:])
                for o in range(2):
                    nc.sync.dma_start(
                        kvsA[:, c, o, :],
                        kvs_tmp[4 * o:4 * o + 4, 0:4096],
                    )

            # ---- kv_t row dots (DVE, overlap AR) ----
            kvt_acc = cpool.tile([128, 8], F32)
            for o in range(2):
                for c in range(3):
                    nc.vector.scalar_tensor_tensor(
                        sq_scratch[:], gB[:, c].rearrange("p w i -> p (w i)"),
                        1.0, wkvt_sb[:, o].rearrange("p w i -> p (w i)"),
                        ALU.mult, ALU.mult,
                        accum_out=kvt_acc[:, 3 * o + c:3 * o + c + 1])
            ps_kvt = pp.tile([6, 32], F32, tag="mm2")
            nc.tensor.matmul(ps_kvt[:], lhsT=kvt_acc[:, 0:6], rhs=qsum_sb[:],
                             start=True, stop=True)
            kvt6 = cpool.tile([6, 32], F32)
            nc.vector.tensor_copy(kvt6[:], ps_kvt[:])
            ktrow = cpool.tile([1, 192], F32)   # (o,c,t)
            nc.sync.dma_start(ktrow[:, 0:192], kvt6[:])

            # ---- post-AR scalar math -> alpha/beta ----
            nc.vector.tensor_scalar_mul(sc2[:, 0:3], sc[:, 24:27], 1.0 / B)
            nc.vector.tensor_scalar_mul(sc2[:, 3:6], sc[:, 27:30], 1.0 / B)
            nc.vector.tensor_mul(sc2[:, 6:9], sc2[:, 0:3], sc2[:, 0:3])
            nc.vector.tensor_sub(sc2[:, 9:12], sc2[:, 3:6], sc2[:, 6:9])
            nc.vector.tensor_scalar(sc2[:, 16:19], sc2[:, 0:3], mw[2],
                                    sc2[:, 26:27], ALU.mult, ALU.add)
            nc.vector.scalar_tensor_tensor(sc2[:, 16:19], sc[:, 16:19], mw[0],
                                           sc2[:, 16:19], ALU.mult, ALU.add)
            nc.vector.tensor_scalar(sc2[:, 20:23], sc2[:, 9:12], vw[2],
                                    sc2[:, 27:28], ALU.mult, ALU.add)
            nc.vector.scalar_tensor_tensor(sc2[:, 20:23], sc[:, 9:12], vw[0],
                                           sc2[:, 20:23], ALU.mult, ALU.add)
            # rstd = exp(-0.5*ln(var+eps)) -> [28:31]
            nc.scalar.activation(sc2[:, 23:26], sc2[:, 20:23], AFT.Ln,
                                 bias=bvals[0:1, 6:7])
            nc.scalar.activation(sc2[:, 28:31], sc2[:, 23:26], AFT.Exp,
                                 scale=-0.5)
            # alpha [0:3], beta [3:6]; bcast row: alpha [6:9], beta*Ws1 [9:12];
            # beta*Wt1 [12:15]
            arow = cpool.tile([1, 32], F32)
            nc.vector.tensor_mul(arow[:, 0:3], sc2[:, 28:31], crow_sb[:, 0:3])
            nc.vector.tensor_mul(arow[:, 3:6], sc2[:, 16:19], arow[:, 0:3])
            nc.vector.tensor_sub(arow[:, 3:6], crow_sb[:, 3:6], arow[:, 3:6])
            nc.vector.tensor_copy(arow[:, 6:9], arow[:, 0:3])
            nc.vector.tensor_mul(arow[:, 9:12], arow[:, 3:6], crow_sb[:, 7:12:2])
            nc.vector.tensor_mul(arow[:, 12:15], arow[:, 3:6], crow_sb[:, 15:18])
            psB = pp.tile([128, 6], F32, tag="mm2")
            nc.tensor.matmul(psB[:], lhsT=ones_row[:], rhs=arow[:, 6:12],
                             start=True, stop=True)

            # ---- As: scale K, exp halves, accumulate softmax sums ----
            ksc = cpool.tile([128, 3, 128], F32)
            nc.vector.tensor_tensor(
                ksc[:], kvsA[:, :, 0, :],
                psB[:, 0:3].unsqueeze(2).broadcast_to([128, 3, 128]), ALU.mult)
            ehalf = cpool.tile([128, 3, 128], F32)
            nc.scalar.activation(ehalf[:], ksc[:], AFT.Exp, scale=0.5)
            red = cpool.tile([128, 4], F32)
            for c in range(3):
                nc.vector.scalar_tensor_tensor(
                    ksc[:, c], ehalf[:, c], 1.0, ehalf[:, c],
                    ALU.mult, ALU.mult, accum_out=red[:, c:c + 1])
            ps_r = pp.tile([1, 3], F32, tag="mm2")
            nc.tensor.matmul(ps_r[:], lhsT=ones_col[:], rhs=red[:, 0:3],
                             start=True, stop=True)
            vfin = cpool.tile([128, 3, 128], F32)
            nc.vector.tensor_tensor(
                vfin[:], kvsA[:, :, 1, :],
                psB[:, 0:3].unsqueeze(2).broadcast_to([128, 3, 128]), ALU.mult)
            nc.vector.tensor_tensor(
                vfin[:], vfin[:],
                psB[:, 3:6].unsqueeze(2).broadcast_to([128, 3, 128]), ALU.add)
            asf = cpool.tile([128, 3, 128], F32)
            nc.vector.tensor_tensor(asf[:], ehalf[:], vfin[:], ALU.mult)

            # ---- At: affine + exp + sums + fold both rsqrt factors ----
            trow = cpool.tile([1, 512], F32)
            nc.vector.tensor_mul(
                ktrow[:, 0:96].rearrange("p (c t) -> p c t", c=3),
                ktrow[:, 0:96].rearrange("p (c t) -> p c t", c=3),
                arow[:, 0:3].unsqueeze(2).broadcast_to([1, 3, 32]))
            nc.vector.tensor_mul(
                ktrow[:, 96:192].rearrange("p (c t) -> p c t", c=3),
                ktrow[:, 96:192].rearrange("p (c t) -> p c t", c=3),
                arow[:, 0:3].unsqueeze(2).broadcast_to([1, 3, 32]))
            nc.vector.tensor_add(
                ktrow[:, 96:192].rearrange("p (c t) -> p c t", c=3),
                ktrow[:, 96:192].rearrange("p (c t) -> p c t", c=3),
                arow[:, 12:15].unsqueeze(2).broadcast_to([1, 3, 32]))
            nc.scalar.activation(trow[:, 0:96], ktrow[:, 0:96], AFT.Exp)
            nc.vector.tensor_reduce(
                trow[:, 96:99], trow[:, 0:96].rearrange("p (c t) -> p c t", c=3),
                AXT.X, ALU.add)
            nc.scalar.activation(trow[:, 128:224], ktrow[:, 0:96], AFT.Exp,
                                 scale=0.5)
            # S = sum_s * sum_t; rfac = rsqrt(S) via Ln/Exp
            nc.vector.tensor_mul(trow[:, 105:108], trow[:, 96:99], ps_r[0:1, 0:3])
            nc.scalar.activation(trow[:, 102:105], trow[:, 105:108], AFT.Ln)
            nc.scalar.activation(trow[:, 108:111], trow[:, 102:105], AFT.Exp,
                                 scale=-0.5)
            nc.vector.tensor_mul(trow[:, 224:320], trow[:, 128:224],
                                 ktrow[:, 96:192])
            nc.vector.tensor_mul(
                trow[:, 224:320].rearrange("p (c t) -> p c t", c=3),
                trow[:, 224:320].rearrange("p (c t) -> p c t", c=3),
                trow[:, 108:111].unsqueeze(2).broadcast_to([1, 3, 32]))
            psB2 = pp.tile([128, 96], F32, tag="mm2")
            nc.tensor.matmul(psB2[:], lhsT=ones_row[:], rhs=trow[:, 224:320],
                             start=True, stop=True)
            atrep = cpool.tile([128, 96], F32)
            nc.vector.tensor_copy(atrep[:], psB2[:])

            # ---- outer product + output DMA ----
            for chunk in range(8):
                t0 = 4 * chunk
                ost = opool.tile([128, 3, 4, 128], F32, tag="ost")
                teng = nc.gpsimd if chunk in (1, 5) else nc.vector
                teng.tensor_tensor(
                    ost[:],
                    asf[:].unsqueeze(2).broadcast_to([128, 3, 4, 128]),
                    atrep[:].rearrange("p (c t) -> p c t", c=3).unsqueeze(3)
                         [:, :, t0:t0 + 4, :].broadcast_to([128, 3, 4, 128]),
                    ALU.mult)
                for c in range(3):
                    if chunk < 4 and c == 2:
                        deng = nc.gpsimd
                    else:
                        deng = nc.sync if (chunk * 3 + c) % 2 == 0 else nc.scalar
                    deng.dma_start(
                        out_d[c, t0:t0 + 4, :, :].transpose([1, 0, 2]),
                        ost[:, c])

    nc.compile()
    return nc


def _in_maps(inputs, consts):
    x = np.asarray(inputs['x'], np.float32)
    xdt = BF16_NP if S1_BF16 else np.float32
    maps = []
    for b in range(N_CORES):
        xp = np.zeros((128, 130, 32), np.float32)
        xp[:, 1:129, :] = x[b, 0].transpose(1, 2, 0)  # [t,h,w] -> [h,w,t]
        maps.append(dict(
            xin=xp.astype(xdt), bandw=consts['bandw'], bandt=consts['bandt'],
            kvs_lhst=consts['kvs_lhst'], qsum=consts['qsum'],
            wkvt4=consts['wkvt4'], crow=consts['crow'],
        ))
    return maps


def kernel(**inputs) -> np.ndarray:
    from concourse.bass_utils import run_bass_kernel_spmd
    consts = _host_constants(inputs)
    nc = build_program(consts['scal'])
    maps = _in_maps(inputs, consts)
    res = run_bass_kernel_spmd(nc, maps, list(range(N_CORES)))
    out = np.stack([res.results[b]['out'] for b in range(N_CORES)], axis=0)
    return out.astype(np.float32)
